# revision 24
# baseline (speedup 1.0000x reference)
"""ApsPool3d TRN2 kernel v10.

Per core (1 batch): input (64, 48, 48, 48) f32 -> output (64, 24, 24, 24) f32.
Pipeline per channel-pair tile (32 tiles, partitions p = c*48+z, free (y,x)):
  DMA in (f32 via SP hwdge; every 4th tile bf16 via gpsimd casting DMA)
  y-blur: 2 flat TT adds (+2 edge-row adds)         [DVE]
  x-blur: 2 flat TT adds + 2 edge-col repairs       [DVE]
  z-blur: 5 matmuls vs block-diag W (bf16)          [PE]
  evac PSUM->stored bf16 (2 half-tiles)             [Act/DVE alternating]
  squares + accum per (2-tile group, yx parity)     [Act]
argmax via P-matmul + reduce + max_index; extraction staged f32 then
predicated DMAs (cond on z-parity) straight to out. STAGE env gates
debug outputs.
"""

import os
import sys

for _p in ("/opt/trn_rl_repo", "/root/.axon_site/_ro/trn_rl_repo"):
    if _p not in sys.path:
        sys.path.insert(0, _p)

import numpy as np

import concourse.bass as bass
import concourse.mybir as mybir
import concourse.tile as tile


# ---- inlined tile_patch ----
def _patched_drain_and_barrier(self, tick_clock, wait_clock):
    nc = self.nc
    carrier = mybir.InstNoOp(
        name="tile_drain_wait_carrier",
        engine=mybir.EngineType.SP,
        ins=[],
        outs=[],
    )
    wait_clock.add_sem_waits(
        carrier, tile.ScopedClock({None: tick_clock.global_clock})
    )
    waits = list(carrier.sync_info.on_wait) if carrier.sync_info else []
    for w in waits:
        nop = nc.sync.nop()
        nsi = nop.ins.sync_info
        if nsi is None:
            nop.ins.sync_info = mybir.SyncInfo(on_wait=[w], on_update=[])
        else:
            nsi.on_wait.append(w)
    nc.sync.drain()
    nc.all_engine_barrier()
    assert self.sems is not None
    popped = nc._tile_sem_poison_stack.pop()
    assert popped is self._sem_poison
    nc.clear_and_free_semaphores(list(self.sems.allocated().values()))
    nc.all_engine_barrier()


tile.TileContext._drain_and_barrier = _patched_drain_and_barrier

_SPLIT_SEQ = [0]


def _split_waits(nc, max_waits=1):
    for f in nc.m.functions:
        for bb in f.blocks:
            new_insts = []
            for inst in bb.instructions:
                si = inst.sync_info
                if si is not None and si.on_wait and len(si.on_wait) > max_waits:
                    waits = list(si.on_wait)
                    keep = waits[:max_waits]
                    extras = waits[max_waits:]
                    del si.on_wait[:]
                    si.on_wait.extend(keep)
                    for w in extras:
                        _SPLIT_SEQ[0] += 1
                        nop = mybir.InstNoOp(
                            name=f"waitsplit-{_SPLIT_SEQ[0]}",
                            engine=inst.engine,
                            ins=[],
                            outs=[],
                            sync_info=mybir.SyncInfo(on_wait=[w], on_update=[]),
                        )
                        new_insts.append(nop)
                new_insts.append(inst)
            if len(new_insts) != len(bb.instructions):
                del bb.instructions[:]
                bb.instructions.extend(new_insts)
# ---- end inlined tile_patch ----

from concourse.bass_utils import run_bass_kernel_spmd

F32 = mybir.dt.float32
BF16 = mybir.dt.bfloat16
FP8 = mybir.dt.float8e4
U32 = mybir.dt.uint32
ALU = mybir.AluOpType

C, N = 64, 48
NH = N // 2  # 24
YX = N * N  # 2304
NT = C // 2  # 32 channel-pair tiles
GROUP = 2  # tiles per norm-square group
NGROUP = NT // GROUP  # 16
EG = 4  # tiles per extraction/staging group
OUTSZ = C * NH * NH * NH

STAGE = int(os.environ.get("STAGE", "5"))
CAST_MOD = int(os.environ.get("CAST_MOD", "2"))  # t % CAST_MOD == CAST_MOD-1 -> gpsimd cast dma
EVAC_DVE_MOD = int(os.environ.get("EVAC_DVE_MOD", "0"))  # t % mod == 1 -> DVE evac


def zperm():
    """j (output partition z-slot) -> z_out. Even z at [0,24), odd at [24,48)."""
    return [2 * i for i in range(NH)] + [2 * i + 1 for i in range(NH)]


def build_selcat():
    """(96, 2x48) bf16: halves for dz=0 / dz=1. Column m = cl*24+z'."""
    sc = np.zeros((96, 96), dtype=np.float32)
    for dz in range(2):
        for cl in range(2):
            for zp in range(NH):
                sc[cl * N + dz * NH + zp, dz * 48 + cl * NH + zp] = 1.0
    return sc


def build_weights(filt):
    """W (96,96) bf16 z-blur with permuted z_out and full 1/64 norm; P (96,2) f32."""
    f = np.asarray(filt[0, 0], dtype=np.float64)
    s = f.sum()  # 64 (pre-normalized to sum 1 -> s=1)
    kz = f.sum(axis=(1, 2)) / s  # [.25,.5,.25]
    zp = zperm()
    blk = np.zeros((N, N), dtype=np.float64)
    for m in range(N):
        z_out = zp[m]
        for dz in (-1, 0, 1):
            z_in = z_out + dz
            if 0 <= z_in < N:
                blk[z_in, m] = kz[dz + 1] / 16.0  # (1/4 y) * (1/4 x)
    W = np.zeros((96, 96), dtype=np.float64)
    for c in range(2):
        W[c * N : (c + 1) * N, c * N : (c + 1) * N] = blk
    P = np.zeros((96, 2), dtype=np.float32)
    for c in range(2):
        P[c * N : c * N + NH, 0] = 1.0
        P[c * N + NH : c * N + N, 1] = 1.0
    return W.astype(np.float32), P


def build_kernel(nc):
    x = nc.declare_dram_parameter("x", [C, N, YX], F32, isOutput=False)
    w_d = nc.declare_dram_parameter("w", [96, 96], BF16, isOutput=False)
    w2_d = nc.declare_dram_parameter("w2", [96, 96], BF16, isOutput=False)
    par_d = nc.declare_dram_parameter("par", [96, 2], F32, isOutput=False)
    selcat_d = nc.declare_dram_parameter("selcat", [96, 96], BF16, isOutput=False)
    out = nc.declare_dram_parameter("out", [C, NH, NH * NH], F32, isOutput=True)
    dbg16 = dbg32 = dbgidx = None
    if STAGE in (1, 2, 3):
        dbg16 = nc.declare_dram_parameter("dbg16", [96, 2 * YX], BF16, isOutput=True)
    if STAGE == 4:
        dbg32 = nc.declare_dram_parameter("dbg32", [1, 8], F32, isOutput=True)
        dbgidx = nc.declare_dram_parameter("dbgidx", [1, 8], U32, isOutput=True)

    with tile.TileContext(nc) as tc:
        with (
            tc.tile_pool(name="consts", bufs=1) as consts,
            tc.tile_pool(name="inp", bufs=1) as inp_pool,
            tc.tile_pool(name="work", bufs=1) as work_pool,
            tc.tile_pool(name="ps", bufs=1, space="PSUM") as psum_pool,
            tc.tile_pool(name="store", bufs=1) as store_pool,
            tc.tile_pool(name="dramp", bufs=1, space="DRAM") as dram_pool,
        ):
            w = consts.tile([96, 96], BF16, tag="w")
            w2 = consts.tile([96, 96], BF16, tag="w2")
            par = consts.tile([96, 2], F32, tag="par")
            selcat = consts.tile([96, 96], BF16, tag="selcat")
            nc.default_dma_engine.dma_start(w[:], w_d[:])
            nc.default_dma_engine.dma_start(w2[:], w2_d[:])
            nc.default_dma_engine.dma_start(par[:], par_d[:])
            nc.default_dma_engine.dma_start(selcat[:], selcat_d[:])

            stored = store_pool.tile([96, NT * YX + 56], BF16, tag="stored")
            norm_acc = consts.tile([128, NGROUP * 4], F32, tag="nacc")

            # rotating buffers
            ins32 = [inp_pool.tile([96, YX], F32, tag=f"i32_{i}", name=f"i32_{i}") for i in range(2)]
            t1 = work_pool.tile([96, YX + N], BF16, tag="t1", name="t1")
            us = [work_pool.tile([96, YX], BF16, tag="u_0", name="u_0")]
            sxs = [work_pool.tile([96, YX], BF16, tag=f"sx_{i}", name=f"sx_{i}") for i in range(2)]
            junk = work_pool.tile([96, GROUP * 576], FP8, tag="junk", name="junk")

            psums = [
                psum_pool.tile([128, 1536], F32, tag=f"ps_{i}", name=f"ps_{i}")
                for i in range(2)
            ]

            for t in range(NT):
                cast = (t % CAST_MOD) == CAST_MOD - 1
                # ---- DMA in ----
                src = x[2 * t : 2 * t + 2].rearrange("c z f -> (c z) f")
                it = ins32[t % 2]
                if cast:
                    d = it[:].bitcast(BF16)[:, 0:YX]
                    nc.gpsimd.dma_start(d, src)
                else:
                    nc.default_dma_engine.dma_start(it[:], src)
                    d = it[:]
                u = us[0][:]
                sx = sxs[t % 2][:]

                # ---- y blur (zero-pad): t1[0]=d[0]; t1[r]=d[r-1]+d[r]; t1[48]=d[47] ----
                nc.vector.tensor_copy(t1[:, 0:N], d[:, 0:N])
                nc.vector.tensor_add(t1[:, N:YX], d[:, 0 : YX - N], d[:, N:YX])
                nc.vector.tensor_copy(t1[:, YX : YX + N], d[:, YX - N : YX])
                # u[y] = t1[y] + t1[y+1], all 48 rows flat
                nc.vector.tensor_add(u, t1[:, 0:YX], t1[:, N : YX + N])

                # ---- x side-sum: sv[a] = u[a-1] + u[a+1]; edge cols repaired ----
                nc.vector.tensor_add(sx[:, 1 : YX - 1], u[:, 0 : YX - 2], u[:, 2:YX])
                uv = u.rearrange("p (y x) -> p y x", x=N)
                svv = sx.rearrange("p (y x) -> p y x", x=N)
                nc.vector.tensor_copy(svv[:, :, 0:1], uv[:, :, 1:2])
                nc.vector.tensor_copy(svv[:, :, N - 1 : N], uv[:, :, N - 2 : N - 1])

                if STAGE == 1:
                    if t == 0:
                        nc.default_dma_engine.dma_start(dbg16[0:96, 0:YX], u[:])
                    if t == 1:
                        nc.default_dma_engine.dma_start(dbg16[0:96, YX : 2 * YX], u[:])
                    continue
                if STAGE == 2:
                    if t == 0:
                        nc.default_dma_engine.dma_start(dbg16[0:96, 0:YX], wt[:])
                    if t == 1:
                        nc.default_dma_engine.dma_start(dbg16[0:96, YX : 2 * YX], wt[:])
                    continue

                # ---- PE: z(x) blur; all u-stream MMs first, then sv-stream ----
                for half in range(2):
                    ps = psums[half]
                    base = half * 1152
                    for ck in range(3):
                        c0 = base + ck * 384
                        nc.tensor.matmul(
                            ps[0:96, ck * 512 : ck * 512 + 384],
                            w2[:], u[:, c0 : c0 + 384], start=True, stop=False,
                        )
                for half in range(2):
                    ps = psums[half]
                    base = half * 1152
                    for ck in range(3):
                        c0 = base + ck * 384
                        nc.tensor.matmul(
                            ps[0:96, ck * 512 : ck * 512 + 384],
                            w[:], sx[:, c0 : c0 + 384], start=False, stop=True,
                        )
                for half in range(2):
                    ps = psums[half]
                    base = half * 1152
                    sview = stored[
                        0:96, t * YX + base : t * YX + base + 1152
                    ].rearrange("p (k f) -> p k f", k=3)
                    pview = ps[0:96].rearrange("p (k f) -> p k f", k=3)[:, :, 0:384]
                    if EVAC_DVE_MOD > 0 and t % EVAC_DVE_MOD == 1:
                        nc.vector.tensor_copy(sview, pview)
                    else:
                        nc.scalar.copy(sview, pview)

                if STAGE == 3:
                    if t == 0:
                        nc.default_dma_engine.dma_start(
                            dbg16[0:96, 0:YX], stored[0:96, 0:YX]
                        )
                    if t == 1:
                        nc.default_dma_engine.dma_start(
                            dbg16[0:96, YX : 2 * YX], stored[0:96, YX : 2 * YX]
                        )

                # ---- norm squares per completed group (Act) ----
                if t % GROUP == GROUP - 1:
                    g = t // GROUP
                    gview = stored[
                        0:96, g * GROUP * YX : (g + 1) * GROUP * YX
                    ].rearrange("p (tt y x) -> p tt y x", tt=GROUP, y=N)
                    jv = junk[:].rearrange("p (tt y x) -> p tt y x", tt=GROUP, y=NH)
                    for pc in range(4):
                        xp, yp = pc >> 1, pc & 1
                        nc.scalar.activation(
                            jv,
                            gview[:, :, yp:N:2, xp:N:2],
                            mybir.ActivationFunctionType.Square,
                            accum_out=norm_acc[0:96, g * 4 + pc : g * 4 + pc + 1],
                        )

            if STAGE <= 2:
                return

            # ---- finalize norms (as baseline) ----
            zred = psum_pool.tile([2, NGROUP * 4], F32, tag="zred")
            nc.tensor.matmul(
                zred[:], par[:, 0:2], norm_acc[0:96, :], start=True, stop=True
            )
            zred_s = consts.tile([2, NGROUP * 4], F32, tag="zreds")
            nc.scalar.copy(zred_s[:], zred[:])
            nbounce = dram_pool.tile([2, 4], F32, tag="nbounce", name="nbounce")
            zv = zred_s[:].rearrange("p (g c) -> p c g", g=NGROUP)
            n8_2 = consts.tile([2, 4], F32, tag="n8_2")
            nc.vector.tensor_reduce(n8_2[:], zv, mybir.AxisListType.X, ALU.add)
            nc.default_dma_engine.dma_start(nbounce[:], n8_2[:])
            norms8 = consts.tile([1, 8], F32, tag="norms8")
            nc.default_dma_engine.dma_start(
                norms8[:],
                nbounce[:].rearrange("z c -> (z c)").rearrange("(o f) -> o f", o=1),
            )
            nmax = consts.tile([1, 8], F32, tag="nmax")
            nidx = consts.tile([1, 8], U32, tag="nidx")
            nc.vector.max(nmax[:], norms8[:])
            nc.vector.max_index(nidx[:], nmax[:], norms8[:])

            if STAGE == 4:
                nc.default_dma_engine.dma_start(dbg32[:], norms8[:])
                nc.default_dma_engine.dma_start(dbgidx[:], nidx[:])
                return

            # ---- registers: phase index -> offsets / conds ----
            rp = nc.alloc_registers("rp")
            ryx = nc.alloc_registers("ryx")
            rz = nc.alloc_registers("rz")
            rtmp = nc.alloc_registers("rtmp")
            nc.regs_load(rp, nidx[0:1, 0:1])
            nc.regs_alu(rtmp, rp, 1, ALU.bitwise_and)  # dy
            nc.regs_alu(ryx, rtmp, N, ALU.mult)  # 48*dy
            nc.regs_alu(rtmp, rp, 1, ALU.logical_shift_right)
            nc.regs_alu(rtmp, rtmp, 1, ALU.bitwise_and)  # dx
            nc.regs_alu(ryx, ryx, rtmp, ALU.add)  # 48*dy + dx
            nc.regs_alu(rtmp, rp, 2, ALU.logical_shift_right)
            nc.regs_alu(rz, rtmp, 1, ALU.bitwise_and)  # dz
            yx_off = nc.snap(ryx, min_val=0, max_val=49)
            rs48 = nc.alloc_registers("rs48")
            nc.regs_alu(rs48, rz, 48, ALU.mult)
            sel48 = nc.snap(rs48, min_val=0, max_val=48)

            # ---- extraction: PE z-select into fixed partitions, direct out ----
            sel_used = consts.tile([96, 48], BF16, tag="selu")
            nc.scalar.copy(sel_used[:], selcat[:, bass.ds(sel48, 48)])
            stgs2 = [
                store_pool.tile([96, EG * 576], F32, tag=f"stgf{i}", name=f"stgf{i}")
                for i in range(2)
            ]
            sbf = [
                store_pool.tile([96, EG * 576], BF16, tag="sbf0", name="sbf0")
            ]
            for g in range(NT // EG):
                src_g = stored[0:96, g * EG * YX : (g + 1) * EG * YX + 56][
                    :, bass.ds(yx_off, EG * YX)
                ].rearrange("p (tt y x) -> p tt y x", tt=EG, y=N)[
                    :, :, 0:N:2, 0:N:2
                ]
                sb = sbf[0]
                stg = stgs2[g % 2]
                dstb = sb[:].rearrange("p (tt y x) -> p tt y x", tt=EG, y=NH)
                if g % 2 == 0:
                    nc.scalar.copy(dstb, src_g)
                else:
                    nc.vector.tensor_copy(dstb, src_g)
                for tt in range(EG):
                    ps = psums[tt % 2]
                    for c0, cw in ((0, 256), (256, 256), (512, 64)):
                        nc.tensor.matmul(
                            ps[0:48, c0 : c0 + cw],
                            sel_used[:],
                            sb[:, tt * 576 + c0 : tt * 576 + c0 + cw],
                            start=True,
                            stop=True,
                        )
                    pv = ps[0:48].rearrange("p (k f) -> p k f", k=3)[:, :, 0:256]
                    hmm = 0
                    dstv = stg[0:48, tt * 576 : tt * 576 + 576]
                    nc.scalar.copy(
                        dstv.rearrange("p (k f) -> p k f", k=3)[:, :, 0:256]
                        if hmm
                        else dstv,
                        ps[0:48, 0:576],
                    )
                for cl in range(2):
                    svd = stg[cl * NH : cl * NH + NH, :].rearrange(
                        "z (tt f) -> z tt f", tt=EG
                    )
                    c0 = 2 * g * EG + cl
                    dd = out[c0 : c0 + 2 * EG - 1 : 2]
                    eng = nc.default_dma_engine if g % 2 == 0 else nc.scalar
                    eng.dma_start(dd.transpose([1, 0, 2]), svd)


_NC_CACHE = {}


def _get_nc():
    if "nc" not in _NC_CACHE:
        nc = bass.Bass()
        build_kernel(nc)
        _split_waits(nc)
        _NC_CACHE["nc"] = nc
    return _NC_CACHE["nc"]


def run(input_to_pool, filt, trace=False):
    import ml_dtypes

    W, P = build_weights(np.asarray(filt))
    nc = _get_nc()
    x = np.ascontiguousarray(np.asarray(input_to_pool, dtype=np.float32))
    B = x.shape[0]
    in_maps = []
    for b in range(B):
        in_maps.append(
            {
                "x": x[b].reshape(C, N, YX),
                "w": W.astype(ml_dtypes.bfloat16),
                "w2": (2.0 * W).astype(ml_dtypes.bfloat16),
                "par": P,
                "selcat": build_selcat().astype(ml_dtypes.bfloat16),
            }
        )
    res = run_bass_kernel_spmd(nc, in_maps, core_ids=list(range(B)), trace=trace)
    outs = np.stack(
        [res.results[b]["out"].reshape(C, NH, NH, NH) for b in range(B)], axis=0
    )
    return outs, res


def kernel(input_to_pool, filt, permute_indices=None):
    """Full-input entry point: (8,64,48,48,48) f32 -> (8,64,24,24,24) f32."""
    outs, _ = run(input_to_pool, filt, trace=False)
    return outs
